# revision 5
# baseline (speedup 1.0000x reference)
"""AttnBlock (LayerNorm + single-head self-attention + proj + residual) on 8
Trainium2 NeuronCores.

Problem: x [4, 512, 64, 64] f32; per batch image: t = LN(x) over channels;
qkv = t @ w_qkv.T; attn = softmax(q k^T / sqrt(c)); out = attn v @ w_proj.T;
y = x + out.

Sharding: 8 cores = 4 batches x 2 query-halves. Each core gets its batch's
full image (token order rolled so its 2048 queries are local tokens 0..2047),
computes LN + K/V over all 4096 tokens and Q over its half, then
scores/softmax/attn-V/proj for its 2048 queries. No collectives.

All heavy matmuls run in fp8e4m3 DoubleRow mode (256-deep contraction per
instruction): every matmul instruction costs ~N_out cycles on the PE
regardless of dtype, so packing 2 contraction rows per partition halves the
instruction count. x arrives packed fp8 from the host ([w, p, r, t] with
channel = 256w+128r+p); weights arrive fp8 pre-scaled by 16 (healthy e4m3
range), the 16s are folded into the exp scale (1/256) and the final proj
eviction scale (1/256).

Layout: everything stays in the transposed [c, token] domain, no on-chip
transposes:
  scoresT[kt, q] = K @ Q^T    (lhsT = K^T chunk, rhs = Q^T chunk)
  outT = V^T @ attnT          (lhsT = V [tok, d] chunk, rhs = E)
  final[q, d] = outT.T @ wprojT
softmax is max-free (LN'd inputs keep scores in ~[-6, 6]); E = exp(s - 2)
fits fp8e4. The denominator is accumulated on DVE in bf16, reduced by a
ones-column matmul, reciprocal'd as a [1, 512] row, broadcast back to all
partitions by a ones-row matmul, and multiplied into the attn numerators at
their PSUM eviction (before proj) - no transpose of the denominator needed.
"""
import numpy as np

import concourse.bass as bass
import concourse.tile as tile
from concourse import mybir
from concourse.bass_utils import run_bass_kernel_spmd

P = 128
C = 512          # channels
T = 4096         # tokens per image
TQ = 2048        # queries per core
CW = 2           # channel pair-groups (C = CW * 2 * P)
TBLK = 512       # token block for LN/QKV phase
NTB = T // TBLK  # 8
NQB = TQ // TBLK  # 4 query blocks
NKT = T // P     # 32 key chunks
F32 = mybir.dt.float32
BF16 = mybir.dt.bfloat16
FP8 = mybir.dt.float8e4
FP = mybir.ActivationFunctionType
DR = mybir.MatmulPerfMode.DoubleRow
SCALE = float(C) ** -0.5
WS = 16.0        # host-side fp8 weight scale


def split_multiwaits(nc, max_waits=1):
    """walrus codegen allows one sync-wait slot on most TPB instruction
    structs; Tile's sem assignment emits several. Split extras into
    wait-only EventSemaphore instructions on the same engine stream."""
    n = 0
    for fn in nc.m.functions:
        for blk in fn.blocks:
            out = []
            for inst in blk.instructions:
                si = inst.sync_info
                if si is not None and si.on_wait is not None and len(si.on_wait) > max_waits:
                    extra = list(si.on_wait[:-max_waits])
                    keep = list(si.on_wait[-max_waits:])
                    for w in extra:
                        ev = mybir.InstEventSemaphore(
                            name=nc.get_next_instruction_name(),
                            engine=inst.engine,
                            sync_info=mybir.SyncInfo(on_wait=[w], on_update=[]),
                        )
                        out.append(ev)
                        n += 1
                    si.on_wait = keep
                out.append(inst)
            blk.instructions[:] = out
    return n


def build_nc():
    nc = bass.Bass()
    xq8 = nc.declare_dram_parameter("xq8", [CW, P, 2, T], FP8, isOutput=False)
    xres = nc.declare_dram_parameter("xres", [TQ, C], F32, isOutput=False)
    wq8 = nc.declare_dram_parameter("wq8", [CW, P, 2, 3 * C], FP8, isOutput=False)
    wp8 = nc.declare_dram_parameter("wp8", [CW, P, 2, C], FP8, isOutput=False)
    gamma = nc.declare_dram_parameter("gamma", [C], F32, isOutput=False)
    beta = nc.declare_dram_parameter("beta", [C], F32, isOutput=False)
    out = nc.declare_dram_parameter("out", [TQ, C], F32, isOutput=True)

    alp = nc.allow_low_precision(reason="fp8/bf16 attention pipeline")
    alp.__enter__()
    with tile.TileContext(nc) as tc:
        with (
            tc.tile_pool(name="xs", bufs=3) as xs,
            tc.tile_pool(name="consts", bufs=1) as consts,
            tc.tile_pool(name="resid", bufs=1) as resid,
        ):
            # prefetch tb=0 x tiles before anything else
            xq0 = []
            for w in range(CW):
                t8 = consts.tile([P, 2, TBLK], FP8, tag=f"xq0{w}", name=f"xq0{w}")
                nc.gpsimd.dma_start(out=t8, in_=xq8[w, :, :, 0:TBLK])
                xq0.append(t8)
            # ---- constants ----
            gcol = []
            bcol = []
            for cc in range(4):
                g = consts.tile([P, 1], F32, tag=f"g{cc}")
                nc.gpsimd.dma_start(
                    out=g, in_=gamma[cc * P:(cc + 1) * P].rearrange("(p o) -> p o", o=1))
                gcol.append(g)
                b = consts.tile([P, 1], F32, tag=f"b{cc}")
                nc.gpsimd.dma_start(
                    out=b, in_=beta[cc * P:(cc + 1) * P].rearrange("(p o) -> p o", o=1))
                bcol.append(b)
            wqt = []   # fp8 qkv weights, packed [128, 2, 1536]
            for w in range(CW):
                t = consts.tile([P, 2, 3 * C], FP8, tag=f"wq{w}", name=f"wq{w}")
                wqt.append(t)
            for lo, hi in ((C, 2 * C), (0, C), (2 * C, 3 * C)):
                for w in range(CW):
                    nc.gpsimd.dma_start(out=wqt[w][:, :, lo:hi],
                                        in_=wq8[w, :, :, lo:hi])
            # dual-fp8 ldweights rejects tiny stationary tiles, so the LN
            # stats matmuls run as plain fp8 with a [P, 1] ones column
            ones8 = consts.tile([P, 1], FP8, tag="ones8")
            nc.vector.memset(ones8, 1.0)
            ones_col_bf = consts.tile([P, 1], BF16, tag="ones_col_bf")
            nc.vector.memset(ones_col_bf, 1.0)
            ones_row = consts.tile([1, P], BF16, tag="ones_row")
            nc.vector.memset(ones_row, 1.0)
            eps_t = consts.tile([1, 1], F32, tag="eps_t")
            nc.vector.memset(eps_t, 1e-5)
            neg2 = consts.tile([P, 1], F32, tag="neg2")
            nc.vector.memset(neg2, -2.0)

            # ---- resident tensors ----
            KT = []   # K^T: 2 x [128, 2, 4096] fp8 (DoubleRow layout, x16)
            for w in range(CW):
                KT.append(resid.tile([P, 2, T], FP8, tag=f"KT{w}", name=f"KT{w}"))
            V = []    # V [tokenpair, d]: 16 x [128, 2, 512] fp8 (x16)
            for u in range(NKT // 2):
                V.append(resid.tile([P, 2, C], FP8, tag=f"V{u}", name=f"V{u}"))
            QP = []   # Q^T resident: [qb][w] -> [128, 2, 512] fp8 (x16)
            for qb in range(NQB):
                QP.append([resid.tile([P, 2, TBLK], FP8, tag=f"QP{qb}_{w}",
                                      name=f"QP{qb}_{w}") for w in range(CW)])

            # =========== Phase B: LN + QKV ===========
            with (
                tc.tile_pool(name="sqs", bufs=2) as sqs,
                tc.tile_pool(name="rows", bufs=2) as rows,
                tc.tile_pool(name="stat", bufs=1) as stat,
                tc.tile_pool(name="lns", bufs=3) as lns,
                tc.tile_pool(name="bcp", bufs=2) as bcp,
                tc.tile_pool(name="ps_bc", bufs=1, space="PSUM") as ps_bc,
                tc.tile_pool(name="ps_qkv", bufs=1, space="PSUM") as ps_qkv,
                tc.tile_pool(name="ps_st", bufs=1, space="PSUM") as ps_st,
            ):
                xq_t = [None] * NTB      # fp8 packed x tiles per block
                rstd_bf = [None] * NTB   # [1, TBLK] bf16
                nmr_bf = [None] * NTB    # [1, TBLK] bf16 (-mu * rstd)
                qkv_slot = [0]

                def qkv_tiles(prefix, tb, n=4):
                    tiles = []
                    for j in range(n):
                        tag = f"pqkv{qkv_slot[0] % 5}"
                        qkv_slot[0] += 1
                        tiles.append(ps_qkv.tile([P, TBLK], F32, tag=tag,
                                                 name=f"{prefix}{tb}_{j}"))
                    return tiles

                def b1_block(tb):
                    ts = slice(tb * TBLK, (tb + 1) * TBLK)
                    if tb == 0:
                        xq = xq0
                    else:
                        xq = []
                        for w in range(CW):
                            t8 = xs.tile([P, 2, TBLK], FP8, tag=f"xq{w}",
                                         name=f"xq{tb}_{w}")
                            nc.gpsimd.dma_start(out=t8, in_=xq8[w, :, :, ts])
                            xq.append(t8)
                    xq_t[tb] = xq
                    # squared copy for the variance sum
                    sq = []
                    for w in range(CW):
                        s8 = sqs.tile([P, 2, TBLK], FP8, tag=f"sq{w}",
                                      name=f"sq{tb}_{w}")
                        nc.scalar.activation(out=s8, in_=xq[w], func=FP.Square)
                        sq.append(s8)
                    # sum and sum-of-squares via ones matmuls (one PSUM bank,
                    # serialized: s1 evicted before s2 starts)
                    s1 = ps_st.tile([1, TBLK], F32, tag="st", name=f"s1_{tb}")
                    for w in range(CW):
                        for r in range(2):
                            nc.tensor.matmul(s1, ones8, xq[w][:, r, :],
                                             start=(w == 0 and r == 0),
                                             stop=(w == CW - 1 and r == 1))
                    mu = rows.tile([1, TBLK], F32, tag="mu", name=f"mu{tb}")
                    nc.scalar.activation(out=mu, in_=s1, func=FP.Copy, scale=1.0 / C)
                    s2 = ps_st.tile([1, TBLK], F32, tag="st", name=f"s2_{tb}")
                    for w in range(CW):
                        for r in range(2):
                            nc.tensor.matmul(s2, ones8, sq[w][:, r, :],
                                             start=(w == 0 and r == 0),
                                             stop=(w == CW - 1 and r == 1))
                    musq = rows.tile([1, TBLK], F32, tag="musq", name=f"musq{tb}")
                    nc.vector.tensor_mul(out=musq, in0=mu, in1=mu)
                    var = rows.tile([1, TBLK], F32, tag="var", name=f"var{tb}")
                    nc.vector.scalar_tensor_tensor(
                        out=var, in0=s2, scalar=1.0 / C, in1=musq,
                        op0=mybir.AluOpType.mult, op1=mybir.AluOpType.subtract)
                    # rstd = exp(-0.5 * ln(var + eps))
                    lnv = rows.tile([1, TBLK], F32, tag="lnv", name=f"lnv{tb}")
                    nc.scalar.activation(out=lnv, in_=var, func=FP.Ln, bias=eps_t)
                    rs = stat.tile([1, TBLK], BF16, tag=f"rs{tb}", name=f"rs{tb}")
                    nc.scalar.activation(out=rs, in_=lnv, func=FP.Exp, scale=-0.5)
                    rstd_bf[tb] = rs
                    nm = stat.tile([1, TBLK], BF16, tag=f"nm{tb}", name=f"nm{tb}")
                    nc.vector.scalar_tensor_tensor(
                        out=nm, in0=mu, scalar=-1.0, in1=rs,
                        op0=mybir.AluOpType.mult, op1=mybir.AluOpType.mult)
                    nmr_bf[tb] = nm

                def b2_block(tb):
                    ts = slice(tb * TBLK, (tb + 1) * TBLK)
                    xq = xq_t[tb]
                    # broadcast rstd / -mu*rstd to all partitions (bf16 copies)
                    bc_r_ps = ps_bc.tile([P, TBLK], F32, tag="bcr", name=f"bcr{tb}")
                    nc.tensor.matmul(bc_r_ps, ones_row, rstd_bf[tb], start=True, stop=True)
                    bc_n_ps = ps_bc.tile([P, TBLK], F32, tag="bcn", name=f"bcn{tb}")
                    nc.tensor.matmul(bc_n_ps, ones_row, nmr_bf[tb], start=True, stop=True)
                    bc_r = bcp.tile([P, TBLK], BF16, tag="bc_r", name=f"bcrc{tb}")
                    nc.scalar.activation(out=bc_r, in_=bc_r_ps, func=FP.Copy)
                    bc_n = bcp.tile([P, TBLK], BF16, tag="bc_n", name=f"bcnc{tb}")
                    nc.vector.tensor_copy(out=bc_n, in_=bc_n_ps)
                    # LN apply, emitted per packed row -> fp8 packed ln tiles
                    lnp = []
                    for w in range(CW):
                        lnp.append(lns.tile([P, 2, TBLK], FP8, tag=f"ln{w}",
                                            name=f"ln{tb}_{w}"))
                    for w in range(CW):
                        for r in range(2):
                            cc = 2 * w + r
                            u = lns.tile([P, TBLK], BF16, tag="u", name=f"u{tb}_{cc}")
                            nc.vector.scalar_tensor_tensor(
                                out=u, in0=xq[w][:, r, :], scalar=gcol[cc], in1=bc_r,
                                op0=mybir.AluOpType.mult, op1=mybir.AluOpType.mult)
                            u2 = lns.tile([P, TBLK], BF16, tag="u2", name=f"u2{tb}_{cc}")
                            nc.vector.scalar_tensor_tensor(
                                out=u2, in0=bc_n, scalar=gcol[cc], in1=u,
                                op0=mybir.AluOpType.mult, op1=mybir.AluOpType.add)
                            nc.scalar.activation(out=lnp[w][:, r, :], in_=u2,
                                                 func=FP.Identity, bias=bcol[cc])
                    # K^T
                    pk = qkv_tiles("pk", tb)
                    for w in range(CW):
                        for dd in range(4):
                            nc.tensor.matmul(
                                pk[dd], wqt[w][:, :, C + dd * P:C + (dd + 1) * P],
                                lnp[w], perf_mode=DR,
                                start=(w == 0), stop=(w == CW - 1))
                    for dd in range(4):
                        kdst = KT[dd // 2][:, dd % 2, ts]
                        if dd % 2 == 0:
                            nc.scalar.activation(out=kdst, in_=pk[dd], func=FP.Copy)
                        else:
                            nc.vector.tensor_copy(out=kdst, in_=pk[dd])
                    # Q^T (local queries only)
                    if tb < NQB:
                        pq = qkv_tiles("pq", tb)
                        for w in range(CW):
                            for dd in range(4):
                                nc.tensor.matmul(
                                    pq[dd], wqt[w][:, :, dd * P:(dd + 1) * P],
                                    lnp[w], perf_mode=DR,
                                    start=(w == 0), stop=(w == CW - 1))
                        for dd in range(4):
                            qdst = QP[tb][dd // 2][:, dd % 2, :]
                            if dd % 2 == 0:
                                nc.scalar.activation(out=qdst, in_=pq[dd], func=FP.Copy)
                            else:
                                nc.vector.tensor_copy(out=qdst, in_=pq[dd])
                    # V (out = [token, d])
                    pv = qkv_tiles("pv", tb)
                    for w in range(CW):
                        for tt in range(4):
                            nc.tensor.matmul(
                                pv[tt], lnp[w][:, :, tt * P:(tt + 1) * P],
                                wqt[w][:, :, 2 * C:3 * C], perf_mode=DR,
                                start=(w == 0), stop=(w == CW - 1))
                    for tt in range(4):
                        g = tb * 4 + tt
                        vdst = V[g // 2][:, g % 2, :]
                        if tt % 2 == 0:
                            nc.scalar.activation(out=vdst, in_=pv[tt], func=FP.Copy)
                        else:
                            nc.vector.tensor_copy(out=vdst, in_=pv[tt])

                LAG = 1
                for step in range(NTB + LAG):
                    if step < NTB:
                        b1_block(step)
                    if step >= LAG:
                        b2_block(step - LAG)

            # proj weights (not needed until phase C)
            wpt = []
            for w in range(CW):
                t = consts.tile([P, 2, C], FP8, tag=f"wp{w}", name=f"wp{w}")
                nc.gpsimd.dma_start(out=t, in_=wp8[w, :, :, :])
                wpt.append(t)
            # =========== Phase C: attention ===========
            with (
                tc.tile_pool(name="es", bufs=8) as es,
                tc.tile_pool(name="outts", bufs=2) as outts,
                tc.tile_pool(name="dens", bufs=2) as dens,
                tc.tile_pool(name="fins", bufs=3) as fins,
                tc.tile_pool(name="xrs", bufs=3) as xrs,
                tc.tile_pool(name="ps_s", bufs=2, space="PSUM") as ps_s,
                tc.tile_pool(name="ps_o", bufs=1, space="PSUM") as ps_o,
                tc.tile_pool(name="ps_aux", bufs=1, space="PSUM") as ps_aux,
                tc.tile_pool(name="ps_pf", bufs=1, space="PSUM") as ps_pf,
            ):
                def make_tail(qb, dacc, po):
                    def tail():
                        # denominator: fold packed rows, partition-reduce,
                        # reciprocal, broadcast back to partitions
                        dsum = dens.tile([P, TBLK], BF16, tag="dsum",
                                         name=f"dsum{qb}")
                        nc.vector.tensor_add(out=dsum, in0=dacc[:, 0, :],
                                             in1=dacc[:, 1, :])
                        pd = ps_aux.tile([1, TBLK], F32, tag="aux", name=f"pd{qb}")
                        nc.tensor.matmul(pd, ones_col_bf, dsum, start=True, stop=True)
                        rec = dens.tile([1, TBLK], BF16, tag="rec", name=f"rec{qb}")
                        nc.vector.reciprocal(out=rec, in_=pd)
                        bc_ps = ps_aux.tile([P, TBLK], F32, tag="aux",
                                            name=f"bcrec{qb}")
                        nc.tensor.matmul(bc_ps, ones_row, rec, start=True, stop=True)
                        bc_rec = dens.tile([P, TBLK], BF16, tag="bc_rec",
                                           name=f"bc_rec{qb}")
                        nc.scalar.activation(out=bc_rec, in_=bc_ps, func=FP.Copy)
                        # normalize numerators at eviction -> packed fp8 outT
                        op8 = []
                        for w in range(CW):
                            op8.append(outts.tile([P, 2, TBLK], FP8, tag=f"op8{w}",
                                                  name=f"op8{qb}_{w}"))
                        for w in range(CW):
                            for r in range(2):
                                nc.vector.tensor_mul(out=op8[w][:, r, :],
                                                     in0=po[2 * w + r], in1=bc_rec)
                        # proj + residual + store
                        for qq in range(4):
                            rows_sl = slice(qb * TBLK + qq * P,
                                            qb * TBLK + (qq + 1) * P)
                            xr = xrs.tile([P, C], F32, tag="xr", name=f"xr{qb}_{qq}")
                            nc.gpsimd.dma_start(out=xr, in_=xres[rows_sl, :])
                            pf = ps_pf.tile([P, C], F32, tag="pf", name=f"pf{qb}_{qq}")
                            for w in range(CW):
                                nc.tensor.matmul(
                                    pf, op8[w][:, :, qq * P:(qq + 1) * P], wpt[w],
                                    perf_mode=DR,
                                    start=(w == 0), stop=(w == CW - 1))
                            fin = fins.tile([P, C], F32, tag="fin", name=f"fin{qb}_{qq}")
                            nc.vector.scalar_tensor_tensor(
                                out=fin, in0=pf, scalar=1.0 / (WS * WS), in1=xr,
                                op0=mybir.AluOpType.mult, op1=mybir.AluOpType.add)
                            nc.gpsimd.dma_start(out=out[rows_sl, :], in_=fin)
                    return tail

                pending_tail = None
                for qb in range(NQB):
                    po = [ps_o.tile([P, TBLK], F32, tag=f"po{cc}", name=f"po{qb}_{cc}")
                          for cc in range(4)]
                    dacc = dens.tile([P, 2, TBLK], BF16, tag="dacc",
                                     name=f"dacc{qb}")
                    pair_t = {}

                    def scores_exp(kt):
                        u = kt // 2
                        if kt % 2 == 0:
                            pair_t[u] = es.tile([P, 2, TBLK], FP8, tag="e",
                                                name=f"e{qb}_{u}")
                        ksl = slice(kt * P, (kt + 1) * P)
                        pscr = ps_s.tile([P, TBLK], F32, tag="pscr",
                                         name=f"pscr{qb}_{kt}")
                        for w in range(CW):
                            nc.tensor.matmul(pscr, KT[w][:, :, ksl], QP[qb][w],
                                             perf_mode=DR,
                                             start=(w == 0), stop=(w == CW - 1))
                        # shifted exp; 1/WS^2 compensates K,Q weight prescale
                        nc.scalar.activation(out=pair_t[u][:, kt % 2, :], in_=pscr,
                                             func=FP.Exp, scale=SCALE / (WS * WS),
                                             bias=neg2)

                    scores_exp(0)
                    for kt in range(NKT):
                        u = kt // 2
                        if kt + 1 < NKT:
                            scores_exp(kt + 1)
                        if kt % 2 == 1:
                            # denominator partial sums (bf16, packed rows)
                            if u == 0:
                                nc.vector.tensor_copy(out=dacc, in_=pair_t[u])
                            else:
                                nc.vector.tensor_add(out=dacc, in0=dacc,
                                                     in1=pair_t[u])
                            for cc in range(4):
                                nc.tensor.matmul(
                                    po[cc], V[u][:, :, cc * P:(cc + 1) * P],
                                    pair_t[u], perf_mode=DR,
                                    start=(u == 0), stop=(u == NKT // 2 - 1))
                        if kt == 6 and pending_tail is not None:
                            pending_tail()
                            pending_tail = None
                    pending_tail = make_tail(qb, dacc, po)
                if pending_tail is not None:
                    pending_tail()
    alp.__exit__(None, None, None)
    split_multiwaits(nc)
    return nc


_NC = None


def _fp8_dtype():
    import ml_dtypes
    return getattr(ml_dtypes, "float8_e4m3", ml_dtypes.float8_e4m3fn)


def _pack_cw(a2d):
    """[512, N] -> [CW, 128, 2, N] with channel = 256w + 128r + p."""
    n = a2d.shape[1]
    return np.ascontiguousarray(a2d.reshape(CW, 2, P, n).transpose(0, 2, 1, 3))


def kernel(x, ln_gamma, ln_beta, w_qkv, w_proj, **run_kwargs):
    global _NC
    f8 = _fp8_dtype()
    x = np.ascontiguousarray(np.asarray(x, dtype=np.float32))
    ln_gamma = np.asarray(ln_gamma, dtype=np.float32)
    ln_beta = np.asarray(ln_beta, dtype=np.float32)
    wq8 = _pack_cw(np.asarray(w_qkv, dtype=np.float32).T * WS).astype(f8)
    wp8 = _pack_cw(np.asarray(w_proj, dtype=np.float32).T * WS).astype(f8)
    b, c, h, w = x.shape
    assert (b, c, h * w) == (4, C, T)

    in_maps = []
    for core in range(8):
        bi, half = core // 2, core % 2
        xt_b = x[bi].reshape(C, T)
        if half == 0:
            xt_i = xt_b
        else:
            xt_i = np.concatenate([xt_b[:, TQ:], xt_b[:, :TQ]], axis=1)
        xres_i = np.ascontiguousarray(xt_i[:, :TQ].T)
        in_maps.append({
            "xq8": _pack_cw(xt_i).astype(f8),
            "xres": xres_i, "wq8": wq8, "wp8": wp8,
            "gamma": ln_gamma, "beta": ln_beta,
        })

    if _NC is None:
        _NC = build_nc()
    res = run_bass_kernel_spmd(_NC, in_maps, core_ids=list(range(8)), **run_kwargs)

    y = np.empty((b, T, C), dtype=np.float32)
    for core in range(8):
        bi, half = core // 2, core % 2
        y[bi, half * TQ:(half + 1) * TQ, :] = res.results[core]["out"]
    y = np.ascontiguousarray(y.transpose(0, 2, 1).reshape(b, C, h, w))
    if run_kwargs:
        return y, res
    return y


# revision 7
# speedup vs baseline: 1.0574x; 1.0574x over previous
"""AttnBlock (LayerNorm + single-head self-attention + proj + residual) on 8
Trainium2 NeuronCores.

Problem: x [4, 512, 64, 64] f32; per batch image: t = LN(x) over channels;
qkv = t @ w_qkv.T; attn = softmax(q k^T / sqrt(c)); out = attn v @ w_proj.T;
y = x + out.

Sharding: 8 cores = 4 batches x 2 query-halves. Each core gets its batch's
full image (token order rolled so its 2048 queries are local tokens 0..2047),
computes LN + K/V over all 4096 tokens and Q over its half, then
scores/softmax/attn-V/proj for its 2048 queries. No collectives.

All heavy matmuls run in fp8e4m3 DoubleRow mode (256-deep contraction per
instruction): every matmul instruction costs ~N_out cycles on the PE
regardless of dtype, so packing 2 contraction rows per partition halves the
instruction count. x arrives packed fp8 from the host ([w, p, r, t] with
channel = 256w+128r+p); weights arrive fp8 pre-scaled by 16 (healthy e4m3
range), the 16s are folded into the exp scale (1/256) and the final proj
eviction scale (1/256).

Layout: everything stays in the transposed [c, token] domain, no on-chip
transposes:
  scoresT[kt, q] = K @ Q^T    (lhsT = K^T chunk, rhs = Q^T chunk)
  outT = V^T @ attnT          (lhsT = V [tok, d] chunk, rhs = E)
  final[q, d] = outT.T @ wprojT
softmax is max-free (LN'd inputs keep scores in ~[-6, 6]); E = exp(s - 2)
fits fp8e4. The denominator is accumulated on DVE in bf16, reduced by a
ones-column matmul, reciprocal'd as a [1, 512] row, broadcast back to all
partitions by a ones-row matmul, and multiplied into the attn numerators at
their PSUM eviction (before proj) - no transpose of the denominator needed.
"""
import numpy as np

import concourse.bass as bass
import concourse.tile as tile
from concourse import mybir
from concourse.bass_utils import run_bass_kernel_spmd

P = 128
C = 512          # channels
T = 4096         # tokens per image
TQ = 2048        # queries per core
CW = 2           # channel pair-groups (C = CW * 2 * P)
TBLK = 512       # token block for LN/QKV phase
NTB = T // TBLK  # 8
NQB = TQ // TBLK  # 4 query blocks
NKT = T // P     # 32 key chunks
F32 = mybir.dt.float32
BF16 = mybir.dt.bfloat16
FP8 = mybir.dt.float8e4
FP = mybir.ActivationFunctionType
DR = mybir.MatmulPerfMode.DoubleRow
SCALE = float(C) ** -0.5
WS = 16.0        # host-side fp8 weight scale


def split_multiwaits(nc, max_waits=1):
    """walrus codegen allows one sync-wait slot on most TPB instruction
    structs; Tile's sem assignment emits several. Split extras into
    wait-only EventSemaphore instructions on the same engine stream."""
    n = 0
    for fn in nc.m.functions:
        for blk in fn.blocks:
            out = []
            for inst in blk.instructions:
                si = inst.sync_info
                if si is not None and si.on_wait is not None and len(si.on_wait) > max_waits:
                    extra = list(si.on_wait[:-max_waits])
                    keep = list(si.on_wait[-max_waits:])
                    for w in extra:
                        ev = mybir.InstEventSemaphore(
                            name=nc.get_next_instruction_name(),
                            engine=inst.engine,
                            sync_info=mybir.SyncInfo(on_wait=[w], on_update=[]),
                        )
                        out.append(ev)
                        n += 1
                    si.on_wait = keep
                out.append(inst)
            blk.instructions[:] = out
    return n


def build_nc():
    nc = bass.Bass()
    xq8 = nc.declare_dram_parameter("xq8", [CW, P, 2, T], FP8, isOutput=False)
    xres = nc.declare_dram_parameter("xres", [TQ, C], F32, isOutput=False)
    wq8 = nc.declare_dram_parameter("wq8", [CW, P, 2, 3 * C], FP8, isOutput=False)
    wp8 = nc.declare_dram_parameter("wp8", [CW, P, 2, C], FP8, isOutput=False)
    gamma = nc.declare_dram_parameter("gamma", [C], F32, isOutput=False)
    beta = nc.declare_dram_parameter("beta", [C], F32, isOutput=False)
    out = nc.declare_dram_parameter("out", [TQ, C], F32, isOutput=True)

    alp = nc.allow_low_precision(reason="fp8/bf16 attention pipeline")
    alp.__enter__()
    with tile.TileContext(nc) as tc:
        with (
            tc.tile_pool(name="xs", bufs=3) as xs,
            tc.tile_pool(name="consts", bufs=1) as consts,
            tc.tile_pool(name="resid", bufs=1) as resid,
        ):
            # prefetch tb=0 x tiles before anything else
            xq0 = []
            for w in range(CW):
                t8 = consts.tile([P, 2, TBLK], FP8, tag=f"xq0{w}", name=f"xq0{w}")
                nc.gpsimd.dma_start(out=t8, in_=xq8[w, :, :, 0:TBLK])
                xq0.append(t8)
            # ---- constants ----
            gcol = []
            bcol = []
            for cc in range(4):
                g = consts.tile([P, 1], F32, tag=f"g{cc}")
                nc.gpsimd.dma_start(
                    out=g, in_=gamma[cc * P:(cc + 1) * P].rearrange("(p o) -> p o", o=1))
                gcol.append(g)
                b = consts.tile([P, 1], F32, tag=f"b{cc}")
                nc.gpsimd.dma_start(
                    out=b, in_=beta[cc * P:(cc + 1) * P].rearrange("(p o) -> p o", o=1))
                bcol.append(b)
            wqt = []   # fp8 qkv weights, packed [128, 2, 1536]
            for w in range(CW):
                t = consts.tile([P, 2, 3 * C], FP8, tag=f"wq{w}", name=f"wq{w}")
                wqt.append(t)
            for lo, hi in ((C, 2 * C), (0, C), (2 * C, 3 * C)):
                for w in range(CW):
                    nc.gpsimd.dma_start(out=wqt[w][:, :, lo:hi],
                                        in_=wq8[w, :, :, lo:hi])
            # dual-fp8 ldweights rejects tiny stationary tiles, so the LN
            # stats matmuls run as plain fp8 with a [P, 1] ones column
            ones8 = consts.tile([P, 1], FP8, tag="ones8")
            nc.vector.memset(ones8, 1.0)
            ones_col_bf = consts.tile([P, 1], BF16, tag="ones_col_bf")
            nc.vector.memset(ones_col_bf, 1.0)
            ones_row = consts.tile([1, P], BF16, tag="ones_row")
            nc.vector.memset(ones_row, 1.0)
            eps_t = consts.tile([1, 1], F32, tag="eps_t")
            nc.vector.memset(eps_t, 1e-5)
            neg2 = consts.tile([P, 1], F32, tag="neg2")
            nc.vector.memset(neg2, -2.0)

            # ---- resident tensors ----
            KT = []   # K^T: 2 x [128, 2, 4096] fp8 (DoubleRow layout, x16)
            for w in range(CW):
                KT.append(resid.tile([P, 2, T], FP8, tag=f"KT{w}", name=f"KT{w}"))
            V = []    # V [tokenpair, d]: 16 x [128, 2, 512] fp8 (x16)
            for u in range(NKT // 2):
                V.append(resid.tile([P, 2, C], FP8, tag=f"V{u}", name=f"V{u}"))
            QP = []   # Q^T resident: [qb][w] -> [128, 2, 512] fp8 (x16)
            for qb in range(NQB):
                QP.append([resid.tile([P, 2, TBLK], FP8, tag=f"QP{qb}_{w}",
                                      name=f"QP{qb}_{w}") for w in range(CW)])

            # =========== Phase B: LN + QKV ===========
            with (
                tc.tile_pool(name="sqs", bufs=2) as sqs,
                tc.tile_pool(name="rows", bufs=2) as rows,
                tc.tile_pool(name="stat", bufs=1) as stat,
                tc.tile_pool(name="lns", bufs=3) as lns,
                tc.tile_pool(name="bcp", bufs=2) as bcp,
                tc.tile_pool(name="ps_bc", bufs=1, space="PSUM") as ps_bc,
                tc.tile_pool(name="ps_qkv", bufs=1, space="PSUM") as ps_qkv,
                tc.tile_pool(name="ps_st", bufs=1, space="PSUM") as ps_st,
            ):
                xq_t = [None] * NTB      # fp8 packed x tiles per block
                rstd_bf = [None] * NTB   # [1, TBLK] bf16
                nmr_bf = [None] * NTB    # [1, TBLK] bf16 (-mu * rstd)
                qkv_slot = [0]

                def qkv_tiles(prefix, tb, n=4):
                    tiles = []
                    for j in range(n):
                        tag = f"pqkv{qkv_slot[0] % 5}"
                        qkv_slot[0] += 1
                        tiles.append(ps_qkv.tile([P, TBLK], F32, tag=tag,
                                                 name=f"{prefix}{tb}_{j}"))
                    return tiles

                def b1_block(tb):
                    ts = slice(tb * TBLK, (tb + 1) * TBLK)
                    if tb == 0:
                        xq = xq0
                    else:
                        xq = []
                        for w in range(CW):
                            t8 = xs.tile([P, 2, TBLK], FP8, tag=f"xq{w}",
                                         name=f"xq{tb}_{w}")
                            nc.gpsimd.dma_start(out=t8, in_=xq8[w, :, :, ts])
                            xq.append(t8)
                    xq_t[tb] = xq
                    # squared copy for the variance sum
                    sq = []
                    for w in range(CW):
                        s8 = sqs.tile([P, 2, TBLK], FP8, tag=f"sq{w}",
                                      name=f"sq{tb}_{w}")
                        nc.scalar.activation(out=s8, in_=xq[w], func=FP.Square)
                        sq.append(s8)
                    # sum and sum-of-squares via ones matmuls (one PSUM bank,
                    # serialized: s1 evicted before s2 starts)
                    s1 = ps_st.tile([1, TBLK], F32, tag="st", name=f"s1_{tb}")
                    for w in range(CW):
                        for r in range(2):
                            nc.tensor.matmul(s1, ones8, xq[w][:, r, :],
                                             start=(w == 0 and r == 0),
                                             stop=(w == CW - 1 and r == 1))
                    mu = rows.tile([1, TBLK], F32, tag="mu", name=f"mu{tb}")
                    nc.scalar.activation(out=mu, in_=s1, func=FP.Copy, scale=1.0 / C)
                    s2 = ps_st.tile([1, TBLK], F32, tag="st", name=f"s2_{tb}")
                    for w in range(CW):
                        for r in range(2):
                            nc.tensor.matmul(s2, ones8, sq[w][:, r, :],
                                             start=(w == 0 and r == 0),
                                             stop=(w == CW - 1 and r == 1))
                    musq = rows.tile([1, TBLK], F32, tag="musq", name=f"musq{tb}")
                    nc.vector.tensor_mul(out=musq, in0=mu, in1=mu)
                    var = rows.tile([1, TBLK], F32, tag="var", name=f"var{tb}")
                    nc.vector.scalar_tensor_tensor(
                        out=var, in0=s2, scalar=1.0 / C, in1=musq,
                        op0=mybir.AluOpType.mult, op1=mybir.AluOpType.subtract)
                    # rstd = exp(-0.5 * ln(var + eps))
                    lnv = rows.tile([1, TBLK], F32, tag="lnv", name=f"lnv{tb}")
                    nc.scalar.activation(out=lnv, in_=var, func=FP.Ln, bias=eps_t)
                    rs = stat.tile([1, TBLK], BF16, tag=f"rs{tb}", name=f"rs{tb}")
                    nc.scalar.activation(out=rs, in_=lnv, func=FP.Exp, scale=-0.5)
                    rstd_bf[tb] = rs
                    nm = stat.tile([1, TBLK], BF16, tag=f"nm{tb}", name=f"nm{tb}")
                    nc.vector.scalar_tensor_tensor(
                        out=nm, in0=mu, scalar=-1.0, in1=rs,
                        op0=mybir.AluOpType.mult, op1=mybir.AluOpType.mult)
                    nmr_bf[tb] = nm

                lnp_t = [None] * NTB

                def b2a_block(tb):
                    xq = xq_t[tb]
                    # broadcast rstd / -mu*rstd to all partitions (bf16 copies)
                    bc_r_ps = ps_bc.tile([P, TBLK], F32, tag="bcr", name=f"bcr{tb}")
                    nc.tensor.matmul(bc_r_ps, ones_row, rstd_bf[tb], start=True, stop=True)
                    bc_n_ps = ps_bc.tile([P, TBLK], F32, tag="bcn", name=f"bcn{tb}")
                    nc.tensor.matmul(bc_n_ps, ones_row, nmr_bf[tb], start=True, stop=True)
                    bc_r = bcp.tile([P, TBLK], BF16, tag="bc_r", name=f"bcrc{tb}")
                    nc.scalar.activation(out=bc_r, in_=bc_r_ps, func=FP.Copy)
                    bc_n = bcp.tile([P, TBLK], BF16, tag="bc_n", name=f"bcnc{tb}")
                    nc.vector.tensor_copy(out=bc_n, in_=bc_n_ps)
                    # LN apply, emitted per packed row -> fp8 packed ln tiles
                    lnp = []
                    for w in range(CW):
                        lnp.append(lns.tile([P, 2, TBLK], FP8, tag=f"ln{w}",
                                            name=f"ln{tb}_{w}"))
                    for w in range(CW):
                        for r in range(2):
                            cc = 2 * w + r
                            u = lns.tile([P, TBLK], BF16, tag="u", name=f"u{tb}_{cc}")
                            nc.vector.scalar_tensor_tensor(
                                out=u, in0=xq[w][:, r, :], scalar=gcol[cc], in1=bc_r,
                                op0=mybir.AluOpType.mult, op1=mybir.AluOpType.mult)
                            u2 = lns.tile([P, TBLK], BF16, tag="u2", name=f"u2{tb}_{cc}")
                            nc.vector.scalar_tensor_tensor(
                                out=u2, in0=bc_n, scalar=gcol[cc], in1=u,
                                op0=mybir.AluOpType.mult, op1=mybir.AluOpType.add)
                            nc.scalar.activation(out=lnp[w][:, r, :], in_=u2,
                                                 func=FP.Identity, bias=bcol[cc])
                    lnp_t[tb] = lnp

                def b2b_block(tb):
                    ts = slice(tb * TBLK, (tb + 1) * TBLK)
                    lnp = lnp_t[tb]
                    # K^T
                    pk = qkv_tiles("pk", tb)
                    for w in range(CW):
                        for dd in range(4):
                            nc.tensor.matmul(
                                pk[dd], wqt[w][:, :, C + dd * P:C + (dd + 1) * P],
                                lnp[w], perf_mode=DR,
                                start=(w == 0), stop=(w == CW - 1))
                    for dd in range(4):
                        kdst = KT[dd // 2][:, dd % 2, ts]
                        if dd % 2 == 0:
                            nc.scalar.activation(out=kdst, in_=pk[dd], func=FP.Copy)
                        else:
                            nc.vector.tensor_copy(out=kdst, in_=pk[dd])
                    # Q^T (local queries only)
                    if tb < NQB:
                        pq = qkv_tiles("pq", tb)
                        for w in range(CW):
                            for dd in range(4):
                                nc.tensor.matmul(
                                    pq[dd], wqt[w][:, :, dd * P:(dd + 1) * P],
                                    lnp[w], perf_mode=DR,
                                    start=(w == 0), stop=(w == CW - 1))
                        for dd in range(4):
                            qdst = QP[tb][dd // 2][:, dd % 2, :]
                            if dd % 2 == 0:
                                nc.scalar.activation(out=qdst, in_=pq[dd], func=FP.Copy)
                            else:
                                nc.vector.tensor_copy(out=qdst, in_=pq[dd])
                    # V (out = [token, d])
                    pv = qkv_tiles("pv", tb)
                    for w in range(CW):
                        for tt in range(4):
                            nc.tensor.matmul(
                                pv[tt], lnp[w][:, :, tt * P:(tt + 1) * P],
                                wqt[w][:, :, 2 * C:3 * C], perf_mode=DR,
                                start=(w == 0), stop=(w == CW - 1))
                    for tt in range(4):
                        g = tb * 4 + tt
                        vdst = V[g // 2][:, g % 2, :]
                        if tt % 2 == 0:
                            nc.scalar.activation(out=vdst, in_=pv[tt], func=FP.Copy)
                        else:
                            nc.vector.tensor_copy(out=vdst, in_=pv[tt])

                # 3-stage software pipeline: stats(n) | bc+LN(n-1) | qkv(n-2)
                for step in range(NTB + 2):
                    if step < NTB:
                        b1_block(step)
                    if 1 <= step < NTB + 1:
                        b2a_block(step - 1)
                    if step >= 2:
                        b2b_block(step - 2)

            # proj weights (not needed until phase C)
            wpt = []
            for w in range(CW):
                t = consts.tile([P, 2, C], FP8, tag=f"wp{w}", name=f"wp{w}")
                nc.gpsimd.dma_start(out=t, in_=wp8[w, :, :, :])
                wpt.append(t)
            # =========== Phase C: attention ===========
            with (
                tc.tile_pool(name="es", bufs=8) as es,
                tc.tile_pool(name="outts", bufs=2) as outts,
                tc.tile_pool(name="dens", bufs=2) as dens,
                tc.tile_pool(name="fins", bufs=3) as fins,
                tc.tile_pool(name="xrs", bufs=3) as xrs,
                tc.tile_pool(name="ps_s", bufs=2, space="PSUM") as ps_s,
                tc.tile_pool(name="ps_o", bufs=1, space="PSUM") as ps_o,
                tc.tile_pool(name="ps_aux", bufs=1, space="PSUM") as ps_aux,
                tc.tile_pool(name="ps_pf", bufs=1, space="PSUM") as ps_pf,
            ):
                def den_chain(qb, dacc, po):
                    # emitted right after the qb's attnV chain: fold packed
                    # denominator rows, partition-reduce, rec = exp(-ln(den))
                    # (scalar engine; DVE reciprocal is 3.3us), broadcast,
                    # normalize the numerators at PSUM eviction
                    dsum = dens.tile([P, TBLK], BF16, tag="dsum",
                                     name=f"dsum{qb}")
                    nc.vector.tensor_add(out=dsum, in0=dacc[:, 0, :],
                                         in1=dacc[:, 1, :])
                    pd = ps_aux.tile([1, TBLK], F32, tag="aux", name=f"pd{qb}")
                    nc.tensor.matmul(pd, ones_col_bf, dsum, start=True, stop=True)
                    lnd = dens.tile([1, TBLK], F32, tag="lnd", name=f"lnd{qb}")
                    nc.scalar.activation(out=lnd, in_=pd, func=FP.Ln)
                    rec = dens.tile([1, TBLK], BF16, tag="rec", name=f"rec{qb}")
                    nc.scalar.activation(out=rec, in_=lnd, func=FP.Exp, scale=-1.0)
                    bc_ps = ps_aux.tile([P, TBLK], F32, tag="aux",
                                        name=f"bcrec{qb}")
                    nc.tensor.matmul(bc_ps, ones_row, rec, start=True, stop=True)
                    bc_rec = dens.tile([P, TBLK], BF16, tag="bc_rec",
                                       name=f"bc_rec{qb}")
                    nc.scalar.activation(out=bc_rec, in_=bc_ps, func=FP.Copy)
                    op8 = []
                    for w in range(CW):
                        op8.append(outts.tile([P, 2, TBLK], FP8, tag=f"op8{w}",
                                              name=f"op8{qb}_{w}"))
                    for w in range(CW):
                        for r in range(2):
                            nc.vector.tensor_mul(out=op8[w][:, r, :],
                                                 in0=po[2 * w + r], in1=bc_rec)
                    return op8

                def make_tail(qb, op8):
                    def tail():
                        # proj + residual + store
                        for qq in range(4):
                            rows_sl = slice(qb * TBLK + qq * P,
                                            qb * TBLK + (qq + 1) * P)
                            xr = xrs.tile([P, C], F32, tag="xr", name=f"xr{qb}_{qq}")
                            nc.gpsimd.dma_start(out=xr, in_=xres[rows_sl, :])
                            pf = ps_pf.tile([P, C], F32, tag="pf", name=f"pf{qb}_{qq}")
                            for w in range(CW):
                                nc.tensor.matmul(
                                    pf, op8[w][:, :, qq * P:(qq + 1) * P], wpt[w],
                                    perf_mode=DR,
                                    start=(w == 0), stop=(w == CW - 1))
                            fin = fins.tile([P, C], F32, tag="fin", name=f"fin{qb}_{qq}")
                            nc.vector.scalar_tensor_tensor(
                                out=fin, in0=pf, scalar=1.0 / (WS * WS), in1=xr,
                                op0=mybir.AluOpType.mult, op1=mybir.AluOpType.add)
                            nc.gpsimd.dma_start(out=out[rows_sl, :], in_=fin)
                    return tail

                pending_tail = None
                for qb in range(NQB):
                    po = [ps_o.tile([P, TBLK], F32, tag=f"po{cc}", name=f"po{qb}_{cc}")
                          for cc in range(4)]
                    dacc = dens.tile([P, 2, TBLK], BF16, tag="dacc",
                                     name=f"dacc{qb}")
                    pair_t = {}

                    def scores_exp(kt):
                        u = kt // 2
                        if kt % 2 == 0:
                            pair_t[u] = es.tile([P, 2, TBLK], FP8, tag="e",
                                                name=f"e{qb}_{u}")
                        ksl = slice(kt * P, (kt + 1) * P)
                        pscr = ps_s.tile([P, TBLK], F32, tag="pscr",
                                         name=f"pscr{qb}_{kt}")
                        for w in range(CW):
                            nc.tensor.matmul(pscr, KT[w][:, :, ksl], QP[qb][w],
                                             perf_mode=DR,
                                             start=(w == 0), stop=(w == CW - 1))
                        # shifted exp; 1/WS^2 compensates K,Q weight prescale
                        nc.scalar.activation(out=pair_t[u][:, kt % 2, :], in_=pscr,
                                             func=FP.Exp, scale=SCALE / (WS * WS),
                                             bias=neg2)

                    scores_exp(0)
                    for kt in range(NKT):
                        u = kt // 2
                        if kt + 1 < NKT:
                            scores_exp(kt + 1)
                        if kt % 2 == 1:
                            # denominator partial sums (bf16, packed rows)
                            if u == 0:
                                nc.vector.tensor_copy(out=dacc, in_=pair_t[u])
                            else:
                                nc.vector.tensor_add(out=dacc, in0=dacc,
                                                     in1=pair_t[u])
                            for cc in range(4):
                                nc.tensor.matmul(
                                    po[cc], V[u][:, :, cc * P:(cc + 1) * P],
                                    pair_t[u], perf_mode=DR,
                                    start=(u == 0), stop=(u == NKT // 2 - 1))
                        if kt == 6 and pending_tail is not None:
                            pending_tail()
                            pending_tail = None
                    op8 = den_chain(qb, dacc, po)
                    pending_tail = make_tail(qb, op8)
                if pending_tail is not None:
                    pending_tail()
    alp.__exit__(None, None, None)
    split_multiwaits(nc)
    return nc


_NC = None


def _fp8_dtype():
    import ml_dtypes
    return getattr(ml_dtypes, "float8_e4m3", ml_dtypes.float8_e4m3fn)


def _pack_cw(a2d):
    """[512, N] -> [CW, 128, 2, N] with channel = 256w + 128r + p."""
    n = a2d.shape[1]
    return np.ascontiguousarray(a2d.reshape(CW, 2, P, n).transpose(0, 2, 1, 3))


def kernel(x, ln_gamma, ln_beta, w_qkv, w_proj, **run_kwargs):
    global _NC
    f8 = _fp8_dtype()
    x = np.ascontiguousarray(np.asarray(x, dtype=np.float32))
    ln_gamma = np.asarray(ln_gamma, dtype=np.float32)
    ln_beta = np.asarray(ln_beta, dtype=np.float32)
    wq8 = _pack_cw(np.asarray(w_qkv, dtype=np.float32).T * WS).astype(f8)
    wp8 = _pack_cw(np.asarray(w_proj, dtype=np.float32).T * WS).astype(f8)
    b, c, h, w = x.shape
    assert (b, c, h * w) == (4, C, T)

    in_maps = []
    for core in range(8):
        bi, half = core // 2, core % 2
        xt_b = x[bi].reshape(C, T)
        if half == 0:
            xt_i = xt_b
        else:
            xt_i = np.concatenate([xt_b[:, TQ:], xt_b[:, :TQ]], axis=1)
        xres_i = np.ascontiguousarray(xt_i[:, :TQ].T)
        in_maps.append({
            "xq8": _pack_cw(xt_i).astype(f8),
            "xres": xres_i, "wq8": wq8, "wp8": wp8,
            "gamma": ln_gamma, "beta": ln_beta,
        })

    if _NC is None:
        _NC = build_nc()
    res = run_bass_kernel_spmd(_NC, in_maps, core_ids=list(range(8)), **run_kwargs)

    y = np.empty((b, T, C), dtype=np.float32)
    for core in range(8):
        bi, half = core // 2, core % 2
        y[bi, half * TQ:(half + 1) * TQ, :] = res.results[core]["out"]
    y = np.ascontiguousarray(y.transpose(0, 2, 1).reshape(b, C, h, w))
    if run_kwargs:
        return y, res
    return y
